# revision 42
# baseline (speedup 1.0000x reference)
"""Trainium2 Bass kernel for nn_CustomConv1d_82085414961669.

The reference "conv" does a row-major reshape of (B, C_in, L_out, K) patches
into rows of length C_in*K, which mixes C_in and L_out. The resulting math
collapses to, for each (b, ci, s) with s = segment of 256 positions:

    out[b, ci, s*256 + co] = bias[co] + sum_t xpad[b, ci, s*256 + t] * M[co, t]

where M[co, t] = sum_k W[co, t-k, k]  (shape 256 x 262), xpad = x padded by 3.

So the whole op is a small GEMM per 256-wide segment, batched over (b, ci, s).
We shard the batch dim across 8 cores (2 per core), build M on the host
(tiny: 256x262), pad/cast x to fp16 on the host, and on each core:
  - DMA-transpose x into SBUF as [t-in-block (128 partitions) x 128-blocks]
  - 3 accumulating matmuls per 128-window tile (contract t in chunks of 128;
    stationary = x-block slice [128t x 128ci], moving = M^T chunk [128t x 256co])
  - DVE adds bias while copying PSUM into a per-(b,h) SBUF staging piece
  - 4 big contiguous output DMAs

Constraint that shaped the structure: walrus allows only ONE sync wait per
instruction, and Tile emits a queue-reuse wait on the 9th+ DMA (8 HW queues,
round-robin). So the kernel issues exactly 7 DMA instructions.
"""

import numpy as np

import concourse.bass as bass
import concourse.mybir as mybir
import concourse.tile as tile
from concourse.bass_utils import run_bass_kernel_spmd
from concourse.vector_clock import ScopedClock


class _SplitDrainTileContext(tile.TileContext):
    """TileContext whose kernel-tail drain is split into single-wait drains.

    The walrus build in this environment allows only one sync wait per
    instruction; TileContext's stock tail emits one drain carrying a wait
    per outstanding processor, which fails codegen ("Too many sync wait
    commands"). Emitting a chain of drains, one wait each, is semantically
    identical (the SP queue executes them in order).
    """

    def _drain_and_barrier(self, tick_clock, wait_clock):
        nc = self.nc
        drain_inst = nc.sync.drain()
        wait_clock.add_sem_waits(
            drain_inst.ins, ScopedClock({None: tick_clock.global_clock})
        )
        si = drain_inst.ins.sync_info
        waits = list(si.on_wait) if si and si.on_wait else []
        if len(waits) > 1:
            drain_inst.ins.sync_info = mybir.SyncInfo(
                on_wait=[waits[0]], on_update=list(si.on_update or [])
            )
            for w in waits[1:]:
                d = nc.sync.drain()
                d.ins.sync_info = mybir.SyncInfo(on_wait=[w], on_update=[])
        nc.all_engine_barrier()
        assert self.sems is not None
        popped = nc._tile_sem_poison_stack.pop()
        assert popped is self._sem_poison
        nc.clear_and_free_semaphores(list(self.sems.allocated().values()))
        nc.all_engine_barrier()

B, C, L = 16, 256, 4096
CO, CI, KW = 256, 256, 7
PAD = 3
NCORES = 8
BPC = B // NCORES  # batches per core
SEG = 256          # output segment width (positions per s)
S = L // SEG       # 16 segments per (b, ci)
T = CI + KW - 1    # 262: contraction length per window
TC = 3             # contraction chunks of 128 (covers t < 384)
LP = (S - 1) * SEG + TC * 128  # 4224 padded length
NJ = LP // 128     # 33 blocks of 128 per (b, ci) row
F16 = mybir.dt.float16
F32 = mybir.dt.float32

_CACHE: dict = {}

# Results of the last run_bass_kernel_spmd call (for test harnesses to read
# exec_time_ns etc. when BASS_TRACE=1).
LAST_RESULTS = None


def _build():
    if "nc" in _CACHE:
        return _CACHE["nc"]
    nc = bass.Bass(
        "TRN2", target_bir_lowering=False, debug=False, num_devices=NCORES
    )
    # x arrives pre-transposed from the host: xt[b, tt, ci*NJ + j] =
    # xpad[b, ci, 128*j + tt]. Plain copy-DMAs load it (the on-chip
    # DMA-transpose path serializes on the single XBAR and must not overlap
    # copy-mode DMAs - known HW hang - so host-side layout prep wins).
    xt = nc.dram_tensor("xt", [BPC, 128, C * NJ], F16, kind="ExternalInput").ap()
    # packed constants: [:, :768] = M^T in 3 chunks of [128, 256] (f16),
    # [:, 768:] = bias as raw f32 bytes viewed as f16 pairs (two copies),
    # replicated across partitions.
    cb = nc.dram_tensor("cb", [128, 1792], F16, kind="ExternalInput").ap()
    out = nc.dram_tensor("out", [BPC, C, L], F32, kind="ExternalOutput").ap()

    with _SplitDrainTileContext(nc) as tc:
        with (
            tc.tile_pool(name="const", bufs=1) as const_pool,
            tc.tile_pool(name="xtp", bufs=1) as xt_pool,
            tc.tile_pool(name="outp", bufs=1) as out_pool,
            tc.tile_pool(name="psum", bufs=8, space="PSUM") as psum_pool,
        ):
            # Const DMA first so the matmuls' weights are resident ~1us in.
            cb_sb = const_pool.tile([128, 1792], F16, tag="cb")
            nc.scalar.dma_start(cb_sb[:], cb)
            mt_sb = cb_sb[:, 0:768].rearrange("p (c n) -> p c n", n=CO)
            bias2_sb = cb_sb[:, 768:1792].bitcast(F32)
            # Absorb the const-DMA wait on DVE's clock before the first real
            # bias-add, keeping every TensorTensor at a single sync wait.
            bias_warm = const_pool.tile([128, 1], F32, tag="bias_warm")
            nc.vector.tensor_copy(bias_warm[:], bias2_sb[:, 0:1])

            # x pieces: batch 0 per ci-half on two queues (first matmuls can
            # start after ~1 MB of input traffic), batch 1 in one piece.
            xt_b0 = [
                xt_pool.tile(
                    [128, 128 * NJ], F16, tag=f"xt_0_{h}", name=f"xt_0_{h}"
                )
                for h in range(2)
            ]
            for h in range(2):
                nc.scalar.dma_start(
                    xt_b0[h][:], xt[0, :, h * 128 * NJ : (h + 1) * 128 * NJ]
                )
            xt_b1 = xt_pool.tile([128, C * NJ], F16, tag="xt_1")
            nc.scalar.dma_start(xt_b1[:], xt[1])

            # ~4.3us of dummy matmuls on the const tile while the x pieces
            # stream in: the PE sits idle anyway and this flips the HAM
            # clock-gate to 2.4 GHz before the real GEMM begins (the gate
            # needs ~3.4us of sustained PE activity; cold matmuls run at
            # half rate).
            for i in range(40):
                ps = psum_pool.tile([128, CO], F32, tag="ps", name=f"warm_{i}")
                nc.tensor.matmul(
                    ps[:],
                    mt_sb[:, 0, 0:128],
                    mt_sb[:, 0, :],
                    start=True,
                    stop=True,
                )

            # window tile = 128 ci x 1 segment; chunk c of window (ci, s)
            # is block j = 2s + c of row ci -> lhsT column stride NJ,
            # a single free dim (walrus requires that for weights APs).
            def gemm_piece(xv_piece, ob_dst):
                """16 segments of one (b, h) piece: 48 matmuls + 16 bias-adds.
                xv_piece: [128, 128 ci, NJ blocks] view; ob_dst: [128, L]."""
                for s in range(S):
                    ps = psum_pool.tile([128, CO], F32)
                    for c in range(TC):
                        nc.tensor.matmul(
                            ps[:],
                            xv_piece[:, :, 2 * s + c],
                            mt_sb[:, c, :],
                            start=(c == 0),
                            stop=(c == TC - 1),
                        )
                    nc.vector.tensor_add(
                        ob_dst[:, s * SEG : (s + 1) * SEG],
                        ps[:],
                        bias2_sb[:, 0:CO],
                    )

            # one output DMA per (b, h) piece: each fires as soon as its 16
            # bias-adds are done, so output traffic streams throughout the
            # GEMM instead of piling up at the end
            xv1 = xt_b1.rearrange("p (ci j) -> p ci j", j=NJ)
            for b in range(BPC):
                for h in range(2):
                    if b == 0:
                        xv = xt_b0[h].rearrange("p (ci j) -> p ci j", j=NJ)
                    else:
                        xv = xv1[:, h * 128 : (h + 1) * 128, :]
                    ob = out_pool.tile(
                        [128, L], F32, tag=f"ob_{b}_{h}", name=f"ob_{b}_{h}"
                    )
                    gemm_piece(xv, ob[:])
                    nc.sync.dma_start(out[b, h * 128 : (h + 1) * 128, :], ob[:])
    _redistribute_matmul_waits(nc)
    _CACHE["nc"] = nc
    return nc


def _redistribute_matmul_waits(nc):
    """Walrus allows one sync wait per instruction. Matmuls that open a
    reused PSUM bank carry two (PE drain of the old group + DVE read done);
    hoist the surplus onto the matmul's preceding zero-wait Ldweights -
    same engine queue, executes immediately before, so ordering semantics
    are identical."""
    for bb in nc.m.functions[0].blocks:
        insts = bb.instructions
        pe_prev = {}
        last_pe = None
        for inst in insts:
            if inst.engine == mybir.EngineType.PE:
                pe_prev[inst.name] = last_pe
                last_pe = inst
        for inst in insts:
            if not isinstance(inst, mybir.InstMatmult):
                continue
            si = inst.sync_info
            if not si or not si.on_wait or len(si.on_wait) <= 1:
                continue
            waits = list(si.on_wait)
            prev = pe_prev.get(inst.name)
            hops = 0
            # Walking a few instructions back on the PE queue is safe: the
            # hoisted waits reference events ~48 matmuls old (PSUM reuse
            # distance), so no dependency cycle can form.
            while len(waits) > 1 and prev is not None and hops < 6:
                hops += 1
                if not isinstance(
                    prev, (mybir.InstLdweights, mybir.InstMatmult)
                ):
                    prev = pe_prev.get(prev.name)
                    continue
                psi = prev.sync_info
                pw = list(psi.on_wait) if psi and psi.on_wait else []
                if len(pw) >= 1:
                    prev = pe_prev.get(prev.name)
                    continue
                pw.append(waits.pop(0))
                prev.sync_info = mybir.SyncInfo(
                    on_wait=pw,
                    on_update=list(psi.on_update) if psi and psi.on_update else [],
                )
                prev = pe_prev.get(prev.name)
            inst.sync_info = mybir.SyncInfo(
                on_wait=waits, on_update=list(si.on_update or [])
            )


def _prep(x, kernel, bias):
    """Host-side shard + layout prep. Returns in_maps for the 8 cores."""
    x = np.ascontiguousarray(np.asarray(x, dtype=np.float32))
    w = np.asarray(kernel, dtype=np.float32)
    bi = np.asarray(bias, dtype=np.float32)

    # M[co, t] = sum_k W[co, t-k, k]
    m = np.zeros((CO, T), dtype=np.float32)
    for k in range(KW):
        m[:, k : k + CI] += w[:, :, k]
    mt = np.zeros((TC * 128, CO), dtype=np.float32)
    mt[:T] = m.T
    mt = mt.reshape(TC, 128, CO).astype(np.float16)

    cb = np.empty((128, 1792), dtype=np.float16)
    cb[:, 0:768] = mt.transpose(1, 0, 2).reshape(128, TC * CO)
    bias2 = np.concatenate([bi, bi]).view(np.float16)
    cb[:, 768:1792] = bias2[None, :]

    xpad = np.zeros((B, C, LP), dtype=np.float16)
    xpad[:, :, PAD : PAD + L] = x
    # pre-transpose per batch: xt[b, tt, ci*NJ + j] = xpad[b, ci, 128j + tt]
    xt = np.ascontiguousarray(
        xpad.reshape(B, C, NJ, 128).transpose(0, 3, 1, 2).reshape(B, 128, C * NJ)
    )

    return [
        {"xt": xt[i * BPC : (i + 1) * BPC], "cb": cb} for i in range(NCORES)
    ]


def kernel(x, kernel, bias):
    global LAST_RESULTS
    nc = _build()
    in_maps = _prep(x, kernel, bias)
    res = run_bass_kernel_spmd(nc, in_maps, core_ids=list(range(NCORES)))
    LAST_RESULTS = res
    return np.concatenate(
        [res.results[i]["out"] for i in range(NCORES)], axis=0
    ).astype(np.float32)


# revision 48
# speedup vs baseline: 1.0036x; 1.0036x over previous
"""Trainium2 Bass kernel for nn_CustomConv1d_82085414961669.

The reference "conv" does a row-major reshape of (B, C_in, L_out, K) patches
into rows of length C_in*K, which mixes C_in and L_out. The resulting math
collapses to, for each (b, ci, s) with s = segment of 256 positions:

    out[b, ci, s*256 + co] = bias[co] + sum_t xpad[b, ci, s*256 + t] * M[co, t]

where M[co, t] = sum_k W[co, t-k, k]  (shape 256 x 262), xpad = x padded by 3.

So the whole op is a small GEMM per 256-wide segment, batched over (b, ci, s).
We shard the batch dim across 8 cores (2 per core), build M on the host
(tiny: 256x262), pad/cast x to fp16 on the host, and on each core:
  - DMA-transpose x into SBUF as [t-in-block (128 partitions) x 128-blocks]
  - 3 accumulating matmuls per 128-window tile (contract t in chunks of 128;
    stationary = x-block slice [128t x 128ci], moving = M^T chunk [128t x 256co])
  - DVE adds bias while copying PSUM into a per-(b,h) SBUF staging piece
  - 4 big contiguous output DMAs

Constraint that shaped the structure: walrus allows only ONE sync wait per
instruction, and Tile emits a queue-reuse wait on the 9th+ DMA (8 HW queues,
round-robin). So the kernel issues exactly 7 DMA instructions.
"""

import numpy as np

import concourse.bass as bass
import concourse.mybir as mybir
import concourse.tile as tile
from concourse.bass_utils import run_bass_kernel_spmd
from concourse.vector_clock import ScopedClock


class _SplitDrainTileContext(tile.TileContext):
    """TileContext whose kernel-tail drain is split into single-wait drains.

    The walrus build in this environment allows only one sync wait per
    instruction; TileContext's stock tail emits one drain carrying a wait
    per outstanding processor, which fails codegen ("Too many sync wait
    commands"). Emitting a chain of drains, one wait each, is semantically
    identical (the SP queue executes them in order).
    """

    def _drain_and_barrier(self, tick_clock, wait_clock):
        nc = self.nc
        drain_inst = nc.sync.drain()
        wait_clock.add_sem_waits(
            drain_inst.ins, ScopedClock({None: tick_clock.global_clock})
        )
        si = drain_inst.ins.sync_info
        waits = list(si.on_wait) if si and si.on_wait else []
        if len(waits) > 1:
            drain_inst.ins.sync_info = mybir.SyncInfo(
                on_wait=[waits[0]], on_update=list(si.on_update or [])
            )
            for w in waits[1:]:
                d = nc.sync.drain()
                d.ins.sync_info = mybir.SyncInfo(on_wait=[w], on_update=[])
        nc.all_engine_barrier()
        assert self.sems is not None
        popped = nc._tile_sem_poison_stack.pop()
        assert popped is self._sem_poison
        nc.clear_and_free_semaphores(list(self.sems.allocated().values()))
        nc.all_engine_barrier()

B, C, L = 16, 256, 4096
CO, CI, KW = 256, 256, 7
PAD = 3
NCORES = 8
BPC = B // NCORES  # batches per core
SEG = 256          # output segment width (positions per s)
S = L // SEG       # 16 segments per (b, ci)
T = CI + KW - 1    # 262: contraction length per window
TC = 3             # contraction chunks of 128 (covers t < 384)
LP = (S - 1) * SEG + TC * 128  # 4224 padded length
NJ = LP // 128     # 33 blocks of 128 per (b, ci) row
F16 = mybir.dt.float16
F32 = mybir.dt.float32

_CACHE: dict = {}

# Results of the last run_bass_kernel_spmd call (for test harnesses to read
# exec_time_ns etc. when BASS_TRACE=1).
LAST_RESULTS = None


def _build():
    if "nc" in _CACHE:
        return _CACHE["nc"]
    nc = bass.Bass(
        "TRN2", target_bir_lowering=False, debug=False, num_devices=NCORES
    )
    # x arrives pre-transposed from the host: xt[b, tt, ci*NJ + j] =
    # xpad[b, ci, 128*j + tt]. Plain copy-DMAs load it (the on-chip
    # DMA-transpose path serializes on the single XBAR and must not overlap
    # copy-mode DMAs - known HW hang - so host-side layout prep wins).
    xt = nc.dram_tensor("xt", [BPC, 128, C * NJ], F16, kind="ExternalInput").ap()
    # packed constants: [:, :768] = M^T in 3 chunks of [128, 256] (f16),
    # [:, 768:] = bias as raw f32 bytes viewed as f16 pairs (two copies),
    # replicated across partitions.
    cb = nc.dram_tensor("cb", [128, 1792], F16, kind="ExternalInput").ap()
    out = nc.dram_tensor("out", [BPC, C, L], F32, kind="ExternalOutput").ap()

    with _SplitDrainTileContext(nc) as tc:
        with (
            tc.tile_pool(name="const", bufs=1) as const_pool,
            tc.tile_pool(name="xtp", bufs=1) as xt_pool,
            tc.tile_pool(name="outp", bufs=1) as out_pool,
            tc.tile_pool(name="psum", bufs=8, space="PSUM") as psum_pool,
        ):
            # Const DMA first so the matmuls' weights are resident ~1us in.
            cb_sb = const_pool.tile([128, 1792], F16, tag="cb")
            nc.scalar.dma_start(cb_sb[:], cb)
            mt_sb = cb_sb[:, 0:768].rearrange("p (c n) -> p c n", n=CO)
            bias2_sb = cb_sb[:, 768:1792].bitcast(F32)
            # Absorb the const-DMA wait on DVE's clock before the first real
            # bias-add, keeping every TensorTensor at a single sync wait.
            bias_warm = const_pool.tile([128, 1], F32, tag="bias_warm")
            nc.vector.tensor_copy(bias_warm[:], bias2_sb[:, 0:1])

            # x pieces: batch 0 per ci-half on two queues (first matmuls can
            # start after ~1 MB of input traffic), batch 1 in one piece.
            xt_b0 = [
                xt_pool.tile(
                    [128, 128 * NJ], F16, tag=f"xt_0_{h}", name=f"xt_0_{h}"
                )
                for h in range(2)
            ]
            for h in range(2):
                nc.scalar.dma_start(
                    xt_b0[h][:], xt[0, :, h * 128 * NJ : (h + 1) * 128 * NJ]
                )
            xt_b1 = xt_pool.tile([128, C * NJ], F16, tag="xt_1")
            nc.scalar.dma_start(xt_b1[:], xt[1])

            # ~4.3us of dummy matmuls on the const tile while the x pieces
            # stream in: the PE sits idle anyway and this flips the HAM
            # clock-gate to 2.4 GHz before the real GEMM begins (the gate
            # needs ~3.4us of sustained PE activity; cold matmuls run at
            # half rate).
            for i in range(40):
                ps = psum_pool.tile([128, CO], F32, tag="ps", name=f"warm_{i}")
                nc.tensor.matmul(
                    ps[:],
                    mt_sb[:, 0, 0:128],
                    mt_sb[:, 0, :],
                    start=True,
                    stop=True,
                )

            # window tile = 128 ci x 1 segment; chunk c of window (ci, s)
            # is block j = 2s + c of row ci -> lhsT column stride NJ,
            # a single free dim (walrus requires that for weights APs).
            def gemm_piece(xv_piece, ob_dst):
                """16 segments of one (b, h) piece: 48 matmuls + 16 bias-adds.
                xv_piece: [128, 128 ci, NJ blocks] view; ob_dst: [128, L]."""
                for s in range(S):
                    ps = psum_pool.tile([128, CO], F32)
                    for c in range(TC):
                        nc.tensor.matmul(
                            ps[:],
                            xv_piece[:, :, 2 * s + c],
                            mt_sb[:, c, :],
                            start=(c == 0),
                            stop=(c == TC - 1),
                        )
                    nc.vector.tensor_add(
                        ob_dst[:, s * SEG : (s + 1) * SEG],
                        ps[:],
                        bias2_sb[:, 0:CO],
                    )

            # one output DMA per (b, h) piece: each fires as soon as its 16
            # bias-adds are done, so output traffic streams throughout the
            # GEMM instead of piling up at the end
            xv1 = xt_b1.rearrange("p (ci j) -> p ci j", j=NJ)
            for b in range(BPC):
                for h in range(2):
                    if b == 0:
                        xv = xt_b0[h].rearrange("p (ci j) -> p ci j", j=NJ)
                    else:
                        xv = xv1[:, h * 128 : (h + 1) * 128, :]
                    ob = out_pool.tile(
                        [128, L], F32, tag=f"ob_{b}_{h}", name=f"ob_{b}_{h}"
                    )
                    gemm_piece(xv, ob[:])
                    nc.sync.dma_start(out[b, h * 128 : (h + 1) * 128, :], ob[:])
    _redistribute_matmul_waits(nc)
    _CACHE["nc"] = nc
    return nc


def _redistribute_matmul_waits(nc):
    """Walrus allows one sync wait per instruction. Matmuls that open a
    reused PSUM bank carry two (PE drain of the old group + DVE read done);
    hoist the surplus onto the matmul's preceding zero-wait Ldweights -
    same engine queue, executes immediately before, so ordering semantics
    are identical."""
    hoistable = (
        mybir.InstMatmult,
        mybir.InstLdweights,
        mybir.InstMemset,
        mybir.InstTensorCopy,
        mybir.InstTensorTensor,
    )
    for bb in nc.m.functions[0].blocks:
        insts = bb.instructions
        pe_prev = {}
        last_by_eng = {}
        for inst in insts:
            pe_prev[inst.name] = last_by_eng.get(inst.engine)
            last_by_eng[inst.engine] = inst
        for inst in insts:
            if not isinstance(inst, (mybir.InstMatmult, mybir.InstTensorTensor)):
                continue
            si = inst.sync_info
            if not si or not si.on_wait or len(si.on_wait) <= 1:
                continue
            waits = list(si.on_wait)
            prev = pe_prev.get(inst.name)
            hops = 0
            # Walking a few instructions back on the PE queue is safe: the
            # hoisted waits reference events ~48 matmuls old (PSUM reuse
            # distance), so no dependency cycle can form.
            while len(waits) > 1 and prev is not None and hops < 6:
                hops += 1
                if not isinstance(prev, hoistable):
                    prev = pe_prev.get(prev.name)
                    continue
                psi = prev.sync_info
                pw = list(psi.on_wait) if psi and psi.on_wait else []
                if len(pw) >= 1:
                    prev = pe_prev.get(prev.name)
                    continue
                pw.append(waits.pop(0))
                prev.sync_info = mybir.SyncInfo(
                    on_wait=pw,
                    on_update=list(psi.on_update) if psi and psi.on_update else [],
                )
                prev = pe_prev.get(prev.name)
            inst.sync_info = mybir.SyncInfo(
                on_wait=waits, on_update=list(si.on_update or [])
            )


def _prep(x, kernel, bias):
    """Host-side shard + layout prep. Returns in_maps for the 8 cores."""
    x = np.ascontiguousarray(np.asarray(x, dtype=np.float32))
    w = np.asarray(kernel, dtype=np.float32)
    bi = np.asarray(bias, dtype=np.float32)

    # M[co, t] = sum_k W[co, t-k, k]
    m = np.zeros((CO, T), dtype=np.float32)
    for k in range(KW):
        m[:, k : k + CI] += w[:, :, k]
    mt = np.zeros((TC * 128, CO), dtype=np.float32)
    mt[:T] = m.T
    mt = mt.reshape(TC, 128, CO).astype(np.float16)

    cb = np.empty((128, 1792), dtype=np.float16)
    cb[:, 0:768] = mt.transpose(1, 0, 2).reshape(128, TC * CO)
    bias2 = np.concatenate([bi, bi]).view(np.float16)
    cb[:, 768:1792] = bias2[None, :]

    xpad = np.zeros((B, C, LP), dtype=np.float16)
    xpad[:, :, PAD : PAD + L] = x
    # pre-transpose per batch: xt[b, tt, ci*NJ + j] = xpad[b, ci, 128j + tt]
    xt = np.ascontiguousarray(
        xpad.reshape(B, C, NJ, 128).transpose(0, 3, 1, 2).reshape(B, 128, C * NJ)
    )

    return [
        {"xt": xt[i * BPC : (i + 1) * BPC], "cb": cb} for i in range(NCORES)
    ]


def kernel(x, kernel, bias):
    global LAST_RESULTS
    nc = _build()
    in_maps = _prep(x, kernel, bias)
    res = run_bass_kernel_spmd(nc, in_maps, core_ids=list(range(NCORES)))
    LAST_RESULTS = res
    return np.concatenate(
        [res.results[i]["out"] for i in range(NCORES)], axis=0
    ).astype(np.float32)
